# revision 35
# baseline (speedup 1.0000x reference)
"""Tensor-parallel causal multi-head attention (RoPE) for 8 Trainium2 NeuronCores.

Problem: B=1, S=2048, D=4096, H=32 heads, head_dim=128, causal, RoPE,
out-projection with bias.  Reference: y = softmax(mask(QK^T/sqrt(hd))) V Wo^T + bo
with Q/K/V = X @ W{q,k,v}^T (nn.Linear convention) and RoPE applied to Q, K.

Sharding: tensor-parallel across heads (4 heads / core) for QKV + attention;
AllToAll re-shards to sequence (256 rows / core) for the out-projection.
Each core returns its 256-row slice of the final output; host concatenates.

v2 vs v1: all matmul operands in bf16 (fp32 PSUM accumulation; RoPE and
softmax normalization arithmetic in fp32 — measured end-to-end rel_l2 ~5e-3
vs the 2e-2 gate).  Q/K/V stay resident in SBUF (no DRAM spill round-trip).
Projections accumulate the full K=4096 contraction in PSUM (no SBUF
quarter-accumulation).  Softmax normalization uses a ones-broadcast matmul
followed by reciprocal_approx_fast (instead of a 3.4us full reciprocal).
Out-projection bias comes from a preloaded fp32 tile (saves 16 matmuls).
Attention q-chunks run largest-first so each head's AllToAll has ~12us of
compute cover before its out-projection consumes the result.
"""

import sys
import numpy as np

for _p in ("/opt/trn_rl_repo",):
    if _p not in sys.path:
        sys.path.insert(0, _p)

B, S, D, H = 1, 2048, 4096, 32
HD = 128          # head dim
NC = 8            # cores
HPC = H // NC     # heads per core = 4
MPC = 3 * HPC     # projection m-tiles per core (Q0..3, K0..3, V0..3) = 12
SQ = S // NC      # seq rows per core after AllToAll = 256
KT = D // 128     # contraction tiles = 32

_cache = {}


def _build_program():
    import concourse.bass as bass
    import concourse.mybir as mybir
    import concourse.tile as tile
    from concourse import bacc
    from contextlib import ExitStack

    F32 = mybir.dt.float32
    F32R = mybir.dt.float32r
    BF16 = mybir.dt.bfloat16
    PF32 = mybir.dt.float32
    AF = mybir.ActivationFunctionType

    nc = bacc.Bacc("TRN2", target_bir_lowering=False, debug=False, num_devices=NC)

    XT = nc.dram_tensor("XT", [D, S], BF16, kind="ExternalInput")
    W4 = nc.dram_tensor("W4", [MPC, 128, KT, 128], BF16, kind="ExternalInput")
    COSQ = nc.dram_tensor("COSQ", [128, S], F32, kind="ExternalInput")
    SINQ = nc.dram_tensor("SINQ", [128, S], F32, kind="ExternalInput")
    COSK = nc.dram_tensor("COSK", [128, S], F32, kind="ExternalInput")
    SINK = nc.dram_tensor("SINK", [128, S], F32, kind="ExternalInput")
    RMAT = nc.dram_tensor("RMAT", [128, 128], F32R, kind="ExternalInput")
    IDN = nc.dram_tensor("IDN", [128, 128], BF16, kind="ExternalInput")
    MASKS = nc.dram_tensor("MASKS", [128, 4, 512], BF16, kind="ExternalInput")
    ONESC = nc.dram_tensor("ONESC", [128, 1], BF16, kind="ExternalInput")
    ONESR = nc.dram_tensor("ONESR", [1, 128], F32R, kind="ExternalInput")
    WOT = nc.dram_tensor("WOT", [HPC, 8, 8, 128, 512], BF16, kind="ExternalInput")
    BOB = nc.dram_tensor("BOB", [128, D], F32, kind="ExternalInput")
    Y = nc.dram_tensor("Y", [SQ, D], F32, kind="ExternalOutput")

    with tile.TileContext(nc) as tc, ExitStack() as top:
        dram = top.enter_context(tc.tile_pool(name="dram", bufs=1, space="DRAM"))
        a2a_in = [dram.tile([NC, HD, SQ], BF16, name=f"a2ai{h}") for h in range(HPC)]
        a2a_out = [dram.tile([NC, HD, SQ], BF16, name=f"a2ao{h}") for h in range(HPC)]
        sync_in = dram.tile([NC, 1, 8], BF16, name="syncin")
        sync_out = dram.tile([NC, 1, 8], BF16, name="syncout")

        # constants via the software-DGE queue (gpsimd): keeps both HWDGE
        # queues clear for the startup X/W stream; none are needed before
        # ~20us in.
        const = top.enter_context(tc.tile_pool(name="const", bufs=1))
        r_sb = const.tile([128, 128], F32R)
        nc.gpsimd.dma_start(r_sb[:], RMAT.ap())
        idn_sb = const.tile([128, 128], BF16)
        nc.gpsimd.dma_start(idn_sb[:], IDN.ap())
        onesc_sb = const.tile([128, 1], BF16)
        nc.gpsimd.dma_start(onesc_sb[:], ONESC.ap())
        onesr_sb = const.tile([1, 128], F32R)
        nc.gpsimd.dma_start(onesr_sb[:], ONESR.ap())

        # Persistent Q^T/K^T ([hd, s], RoPE'd) and V ([s%128, s//128, hd]).
        res = top.enter_context(tc.tile_pool(name="res", bufs=1))
        qk_res = [res.tile([128, S], BF16, name=f"qk{m}") for m in range(8)]
        v_res = [res.tile([128, S // 128, 128], BF16, name=f"v{h}") for h in range(HPC)]

        XT_t = XT.ap().rearrange("(k p) s -> p k s", p=128)

        # ---- Phase 1: QKV projections + RoPE (Q,K) + transpose (V) ----
        # s-chunk outer (4 x 512 cols), m-tile inner; the full K=4096
        # contraction accumulates in one PSUM bank (32 matmuls).  X streams
        # once (double-buffered 4MB chunks), W streams once per chunk.
        with nc.named_scope("proj"), ExitStack() as ph:
            xt_pool = ph.enter_context(tc.tile_pool(name="xt", bufs=2))
            w_pool = ph.enter_context(tc.tile_pool(name="w", bufs=4))
            cs_pool = ph.enter_context(tc.tile_pool(name="cossin", bufs=2))
            rtmp = ph.enter_context(tc.tile_pool(name="rtmp", bufs=2))
            full_pool = ph.enter_context(tc.tile_pool(name="full", bufs=2))
            ppsum = ph.enter_context(tc.tile_pool(name="ppsum", bufs=4, space="PSUM"))
            rpsum = ph.enter_context(tc.tile_pool(name="rpsum", bufs=2, space="PSUM"))

            def load_cs(a):
                # scalar-queue DMAs: second HWDGE queue, decoupled from the
                # W-slab stream on the sync queue (ACT has slack in proj).
                cst = {}
                for nm, tt in (("cq", COSQ), ("sq", SINQ), ("ck", COSK), ("sk", SINK)):
                    t = cs_pool.tile([128, 512], F32, name=nm)
                    nc.scalar.dma_start(t[:], tt.ap()[:, a * 512 : (a + 1) * 512])
                    cst[nm] = t
                return cst

            def evacuate(m, a, ps, cst):
                s0 = a * 512
                sl = slice(s0, s0 + 512)
                if m < 8:  # Q or K m-tile: RoPE -> qk_res[m]
                    isq = m < 4
                    cos_t = cst["cq" if isq else "ck"]
                    sin_t = cst["sq" if isq else "sk"]
                    full_sb = full_pool.tile([128, 512], F32R, name="full")
                    nc.scalar.copy(full_sb[:], ps[:])
                    # rotate-half as a partition swap via SBUF->SBUF DMA
                    # (sign is folded into SINQ/SINK host-side) — keeps the
                    # rotation off the PE (was a 548ns fp32 matmul each).
                    rot_sb = rtmp.tile([128, 512], F32R, name="rot")
                    nc.scalar.dma_start(rot_sb[0:64, :], full_sb[64:128, :])
                    nc.sync.dma_start(rot_sb[64:128, :], full_sb[0:64, :])
                    c1 = rtmp.tile([128, 512], F32, name="c1")
                    nc.vector.tensor_mul(c1[:], full_sb[:], cos_t[:])
                    r1 = rtmp.tile([128, 512], F32, name="r1")
                    nc.vector.tensor_mul(r1[:], rot_sb[:], sin_t[:])
                    nc.vector.tensor_add(qk_res[m][:, sl], c1[:], r1[:])
                else:  # V m-tile: transpose to [s, hd] -> v_res[h]
                    h = m - 8
                    v_full = full_pool.tile([128, 512], BF16, name="vfull")
                    nc.scalar.copy(v_full[:], ps[:])
                    tps = rpsum.tile([128, 512], BF16, name="vtr")
                    for j in range(4):
                        jj = slice(j * 128, (j + 1) * 128)
                        nc.tensor.transpose(tps[:, jj], v_full[:, jj], idn_sb[:])
                    nc.vector.tensor_copy(v_res[h][:, 4 * a : 4 * a + 4, :], tps[:])

            def load_w(m):
                w_sb = w_pool.tile([128, KT, 128], BF16, name="w")
                nc.sync.dma_start(w_sb[:], W4.ap()[m])
                return w_sb

            # Startup interleave: first W slab goes out first (the first
            # matmul chain needs it plus only the first X piece), X pieces
            # pace the first chain, later W slabs slot between pieces.
            kq = KT // 8
            xt_cur = xt_pool.tile([128, KT, 512], BF16, name="xt")

            def xt0_piece(s, eng):
                eng.dma_start(xt_cur[:, s * kq : (s + 1) * kq, :],
                              XT_t[:, s * kq : (s + 1) * kq, 0:512])

            # startup: chunk-0 pieces split across both HWDGE queues so the
            # 4MB load lands in ~half the serial time; W slabs interleave
            # on the sync queue.
            w_pre = [load_w(0)]
            xt0_piece(0, nc.scalar)
            xt0_piece(1, nc.sync)
            xt0_piece(2, nc.scalar)
            w_pre.append(load_w(4))
            xt0_piece(4, nc.scalar)
            xt0_piece(3, nc.sync)
            w_pre.append(load_w(8))
            xt0_piece(5, nc.scalar)
            xt0_piece(6, nc.scalar)
            xt0_piece(7, nc.scalar)
            xt_next, cs_next = None, None
            cs_cur = load_cs(0)
            for a in range(4):
                for mi, m in enumerate((0, 4, 8, 1, 5, 9, 2, 6, 10, 3, 7, 11)):
                    w_sb = w_pre[mi] if (a == 0 and mi < 3) else load_w(m)
                    if a < 3:
                        # spread next-chunk prefetch: one 512KB piece per m
                        if mi == 2:
                            xt_next = xt_pool.tile([128, KT, 512], BF16,
                                                   name="xt")
                        if 2 <= mi < 10:
                            s = mi - 2
                            nc.scalar.dma_start(
                                xt_next[:, s * kq : (s + 1) * kq, :],
                                XT_t[:, s * kq : (s + 1) * kq,
                                     (a + 1) * 512 : (a + 2) * 512],
                            )
                        if mi == 10:
                            cs_next = load_cs(a + 1)
                    ps = ppsum.tile([128, 512], PF32, name="proj")
                    for kt in range(KT):
                        nc.tensor.matmul(ps[:], w_sb[:, kt, :], xt_cur[:, kt, :],
                                         start=(kt == 0), stop=(kt == KT - 1))
                    evacuate(m, a, ps, cs_cur)
                    if a == 0 and mi == 0:
                        # tiny collective doubling as a cross-core barrier:
                        # absorbs SPMD launch skew here, where the PE still
                        # has ~400us of queued work, so the first real
                        # AllToAll is not a skew sink.
                        zz = full_pool.tile([1, NC * 8], BF16, name="zz")
                        nc.any.memset(zz[:], 0.0)
                        nc.sync.dma_start(sync_in.rearrange("i p q -> p (i q)"),
                                          zz[:])
                        nc.gpsimd.collective_compute(
                            "AllToAll",
                            mybir.AluOpType.bypass,
                            replica_groups=[list(range(NC))],
                            ins=[sync_in.opt()],
                            outs=[sync_out.opt()],
                        )
                xt_cur, cs_cur = xt_next, cs_next

        # ---- Phase 2+3: attention interleaved with out-projection ----
        with nc.named_scope("attn"), ExitStack() as ph:
            es_pool = ph.enter_context(tc.tile_pool(name="es", bufs=6))
            on_pool = ph.enter_context(tc.tile_pool(name="on", bufs=4))
            of_pool = ph.enter_context(tc.tile_pool(name="of", bufs=2))
            wo_pool = ph.enter_context(tc.tile_pool(name="wo", bufs=6))
            yac_pool = ph.enter_context(tc.tile_pool(name="yac", bufs=1))
            yev_pool = ph.enter_context(tc.tile_pool(name="yev", bufs=4))
            msk_pool = ph.enter_context(tc.tile_pool(name="msk", bufs=1))
            # scores, oproj chains and the colsum broadcast share one 5-slot
            # pool (same tag): pure-attention stretches get a 5-deep score
            # pipeline (hides the exp latency), od bursts still get 2 slots.
            spsum = ph.enter_context(tc.tile_pool(name="spsum", bufs=5, space="PSUM"))
            opsum = ph.enter_context(tc.tile_pool(name="opsum", bufs=2, space="PSUM"))
            cpsum = ph.enter_context(tc.tile_pool(name="cpsum", bufs=1, space="PSUM"))
            aux_ps = spsum

            masks_sb = msk_pool.tile([128, 4, 512], BF16, name="masks")
            nc.sync.dma_start(masks_sb[:], MASKS.ap())
            bias_sb = msk_pool.tile([128, D], F32, name="bias")
            nc.sync.dma_start(bias_sb[:], BOB.ap())
            yac = [yac_pool.tile([128, D], F32, name=f"yac{q2}") for q2 in range(2)]
            of_tiles = {}

            def attn_qr(h, qr):
                nk = 4 * (qr + 1)
                qsl = slice(qr * 512, (qr + 1) * 512)
                ops = opsum.tile([128, 512], PF32, name="ot")
                cps = cpsum.tile([1, 512], PF32, name="cs")
                for kt in range(nk):
                    sps = spsum.tile([128, 512], PF32, name="ps")
                    nc.tensor.matmul(
                        sps[:], qk_res[4 + h][:, kt * 128 : (kt + 1) * 128],
                        qk_res[h][:, qsl], start=True, stop=True,
                    )
                    es = es_pool.tile([128, 512], BF16, name="es")
                    nc.scalar.activation(es[:], sps[:], AF.Exp)
                    if kt >= 4 * qr:
                        j = kt - 4 * qr
                        nc.vector.tensor_mul(es[:], es[:], masks_sb[:, j, :])
                    nc.tensor.matmul(ops[:], v_res[h][:, kt, :], es[:],
                                     start=(kt == 0), stop=(kt == nk - 1))
                    nc.tensor.matmul(cps[:], onesc_sb[:], es[:],
                                     start=(kt == 0), stop=(kt == nk - 1))
                # normalization: broadcast colsum across partitions, then
                # fast-approx reciprocal on all 128 lanes.
                cs_sb2 = on_pool.tile([1, 512], F32R, name="css")
                nc.vector.tensor_copy(cs_sb2[:], cps[:])
                bps = aux_ps.tile([128, 512], PF32, name="ps")
                nc.tensor.matmul(bps[:], onesr_sb[:], cs_sb2[:],
                                 start=True, stop=True)
                rec_sb = on_pool.tile([128, 512], F32, name="rec")
                nc.vector.reciprocal_approx_fast(out=rec_sb[:], in_=bps[:])
                otn = on_pool.tile([128, 512], BF16, name="otn")
                nc.vector.tensor_mul(otn[:], ops[:], rec_sb[:])
                for half in range(2):
                    nc.sync.dma_start(
                        a2a_in[h][2 * qr + half, :, :],
                        otn[:, half * 256 : (half + 1) * 256],
                    )

            def attn_head_end(h):
                nc.gpsimd.collective_compute(
                    "AllToAll",
                    mybir.AluOpType.bypass,
                    replica_groups=[list(range(NC))],
                    ins=[a2a_in[h].opt()],
                    outs=[a2a_out[h].opt()],
                )
                of = of_pool.tile([128, 8, SQ], BF16, name="of")
                nc.sync.dma_start(of[:], a2a_out[h].rearrange("i p q -> p i q"))
                of_tiles[h] = of

            def oproj_od(h, od):
                osl = slice(od * 512, (od + 1) * 512)
                wos = []
                for ih in range(2):
                    wo_sb = wo_pool.tile([128, 4, 512], BF16, name="wo")
                    # split the wo stream across both HWDGE queues: the sync
                    # queue alone starves the out-projection tail.
                    eng = nc.sync if ih == 0 else nc.scalar
                    eng.dma_start(
                        wo_sb[:],
                        WOT.ap()[h, od, ih * 4 : (ih + 1) * 4].rearrange(
                            "i p c -> p i c"
                        ),
                    )
                    wos.append(wo_sb)
                for q2 in range(2):
                    q2sl = slice(q2 * 128, (q2 + 1) * 128)
                    ps = aux_ps.tile([128, 512], PF32, name="ps")
                    for i in range(8):
                        nc.tensor.matmul(
                            ps[:], of_tiles[h][:, i, q2sl], wos[i // 4][:, i % 4, :],
                            start=(i == 0), stop=(i == 7),
                        )
                    if h == 0:
                        nc.vector.tensor_add(yac[q2][:, osl], ps[:],
                                             bias_sb[:, osl])
                    elif h < 3:
                        nc.vector.tensor_add(yac[q2][:, osl], yac[q2][:, osl],
                                             ps[:])
                    else:
                        y_sb = yev_pool.tile([128, 512], F32, name="y")
                        nc.vector.tensor_add(y_sb[:], ps[:], yac[q2][:, osl])
                        # Y stores ride the software-DGE queue: on a HWDGE
                        # queue they head-of-line block the od(3) wo loads
                        # (they wait on data that waits on the last a2a).
                        nc.gpsimd.dma_start(Y.ap()[q2sl, osl], y_sb[:])

            # qr largest-first.  Head h-1's out-projection slices start only
            # after qr3+qr2 of head h (~22us of compute cover for the
            # AllToAll, which doubles as a cross-core sync barrier), and two
            # slices trail past attn_head_end(h) so the next AllToAll always
            # has PE work to hide behind.
            # Out-projection lags the AllToAll by a FULL head (h-2 consumed
            # during head h): the Tile scheduler hoists every ready matmul,
            # so anything less leaves the oproj head-of-line-blocked on the
            # collective.  od slices of head 2 likewise cover a2a(3).
            for h in range(HPC):
                for i, qr in enumerate((3, 2, 1, 0)):
                    attn_qr(h, qr)
                    if h >= 2:
                        oproj_od(h - 2, 2 * i)
                        oproj_od(h - 2, 2 * i + 1)
                attn_head_end(h)
            for od in range(8):
                oproj_od(2, od)
            for od in range(8):
                oproj_od(3, od)

    nc.compile()
    return nc


def _prep_inputs(X, Wq, Wk, Wv, Wo, bo, cos, sin):
    import ml_dtypes
    BF = ml_dtypes.bfloat16

    X = np.asarray(X, dtype=np.float32)
    cos = np.asarray(cos, dtype=np.float32)
    sin = np.asarray(sin, dtype=np.float32)

    XTn = np.ascontiguousarray(X.reshape(S, D).T).astype(BF)   # [D, S]
    cosT = np.ascontiguousarray(cos.T)                         # [128, S]
    sinT = np.ascontiguousarray(sin.T).copy()
    # fold the rotate-half sign into sin: rope = x*cos + swap(x)*sin'
    # where swap is a pure partition exchange and sin'[:64] = -sin[:64].
    sinT[0:64, :] *= -1.0
    scale = np.float32(1.0 / np.sqrt(HD))
    COSQ, SINQ = cosT * scale, sinT * scale
    COSK, SINK = cosT, sinT

    R = np.zeros((128, 128), np.float32)
    for hd in range(64):
        R[hd + 64, hd] = -1.0     # rot[hd] = -x[hd+64]
        R[hd, hd + 64] = 1.0      # rot[hd+64] = x[hd]
    IDN = np.eye(128, dtype=np.float32).astype(BF)

    # masks[k, j, q] = 1 if (j*128 + k) <= q  (within a diagonal 512-block)
    kk = np.arange(128)[:, None, None]
    jj = np.arange(4)[None, :, None]
    qq = np.arange(512)[None, None, :]
    MASKS = ((jj * 128 + kk) <= qq).astype(np.float32).astype(BF)

    ONESC = np.ones((128, 1), np.float32).astype(BF)
    ONESR = np.ones((1, 128), np.float32)

    # [h, od, i, p, c] with global k-tile = 4*i + h (source core i, head h)
    WoT8 = np.ascontiguousarray(
        np.asarray(Wo, np.float32)
        .reshape(8, 512, 8, HPC, 128)
        .transpose(3, 0, 2, 4, 1)
    ).astype(BF)
    BOB = np.broadcast_to(np.asarray(bo, np.float32)[None, :], (128, D)).copy()

    shared = dict(
        XT=XTn, COSQ=COSQ, SINQ=SINQ, COSK=COSK, SINK=SINK, RMAT=R, IDN=IDN,
        MASKS=MASKS, ONESC=ONESC, ONESR=ONESR, WOT=WoT8, BOB=BOB,
    )
    in_maps = []
    for c in range(NC):
        lo, hi = c * 512, (c + 1) * 512
        Wcat = np.concatenate(
            [np.asarray(Wq, np.float32)[lo:hi],
             np.asarray(Wk, np.float32)[lo:hi],
             np.asarray(Wv, np.float32)[lo:hi]], axis=0
        )  # [1536, D]
        W4 = np.ascontiguousarray(
            Wcat.reshape(MPC, 128, KT, 128).transpose(0, 3, 2, 1)
        ).astype(BF)  # [m, p(k), k-tile, c(out)]
        in_maps.append({**shared, "W4": W4})
    return in_maps


def kernel(X, Wq, Wk, Wv, Wo, bo, cos, sin, _trace=False):
    from concourse.bass_utils import run_bass_kernel_spmd

    if "nc" not in _cache:
        _cache["nc"] = _build_program()
    nc = _cache["nc"]

    in_maps = _prep_inputs(X, Wq, Wk, Wv, Wo, bo, cos, sin)
    res = run_bass_kernel_spmd(nc, in_maps, list(range(NC)), trace=_trace)
    _cache["last_result"] = res
    Yfull = np.concatenate([res.results[c]["Y"] for c in range(NC)], axis=0)
    return Yfull.reshape(B, S, D).astype(np.float32)


# revision 39
# speedup vs baseline: 1.0241x; 1.0241x over previous
"""Tensor-parallel causal multi-head attention (RoPE) for 8 Trainium2 NeuronCores.

Problem: B=1, S=2048, D=4096, H=32 heads, head_dim=128, causal, RoPE,
out-projection with bias.  Reference: y = softmax(mask(QK^T/sqrt(hd))) V Wo^T + bo
with Q/K/V = X @ W{q,k,v}^T (nn.Linear convention) and RoPE applied to Q, K.

Sharding: tensor-parallel across heads (4 heads / core) for QKV + attention;
AllToAll re-shards to sequence (256 rows / core) for the out-projection.
Each core returns its 256-row slice of the final output; host concatenates.

v2 vs v1: all matmul operands in bf16 (fp32 PSUM accumulation; RoPE and
softmax normalization arithmetic in fp32 — measured end-to-end rel_l2 ~5e-3
vs the 2e-2 gate).  Q/K/V stay resident in SBUF (no DRAM spill round-trip).
Projections accumulate the full K=4096 contraction in PSUM (no SBUF
quarter-accumulation).  Softmax normalization uses a ones-broadcast matmul
followed by reciprocal_approx_fast (instead of a 3.4us full reciprocal).
Out-projection bias comes from a preloaded fp32 tile (saves 16 matmuls).
Attention q-chunks run largest-first so each head's AllToAll has ~12us of
compute cover before its out-projection consumes the result.
"""

import sys
import numpy as np

for _p in ("/opt/trn_rl_repo",):
    if _p not in sys.path:
        sys.path.insert(0, _p)

B, S, D, H = 1, 2048, 4096, 32
HD = 128          # head dim
NC = 8            # cores
HPC = H // NC     # heads per core = 4
MPC = 3 * HPC     # projection m-tiles per core (Q0..3, K0..3, V0..3) = 12
SQ = S // NC      # seq rows per core after AllToAll = 256
KT = D // 128     # contraction tiles = 32

_cache = {}


def _build_program():
    import concourse.bass as bass
    import concourse.mybir as mybir
    import concourse.tile as tile
    from concourse import bacc
    from contextlib import ExitStack

    F32 = mybir.dt.float32
    F32R = mybir.dt.float32r
    BF16 = mybir.dt.bfloat16
    PF32 = mybir.dt.float32
    AF = mybir.ActivationFunctionType

    nc = bacc.Bacc("TRN2", target_bir_lowering=False, debug=False, num_devices=NC)

    XT = nc.dram_tensor("XT", [D, S], BF16, kind="ExternalInput")
    W4 = nc.dram_tensor("W4", [MPC, 128, KT, 128], BF16, kind="ExternalInput")
    COSQ = nc.dram_tensor("COSQ", [128, S], F32, kind="ExternalInput")
    SINQ = nc.dram_tensor("SINQ", [128, S], F32, kind="ExternalInput")
    COSK = nc.dram_tensor("COSK", [128, S], F32, kind="ExternalInput")
    SINK = nc.dram_tensor("SINK", [128, S], F32, kind="ExternalInput")
    RMAT = nc.dram_tensor("RMAT", [128, 128], F32R, kind="ExternalInput")
    IDN = nc.dram_tensor("IDN", [128, 128], BF16, kind="ExternalInput")
    MASKS = nc.dram_tensor("MASKS", [128, 4, 512], BF16, kind="ExternalInput")
    ONESC = nc.dram_tensor("ONESC", [128, 1], BF16, kind="ExternalInput")
    ONESR = nc.dram_tensor("ONESR", [1, 128], F32R, kind="ExternalInput")
    WOT = nc.dram_tensor("WOT", [HPC, 8, 8, 128, 512], BF16, kind="ExternalInput")
    BOB = nc.dram_tensor("BOB", [128, D], F32, kind="ExternalInput")
    Y = nc.dram_tensor("Y", [SQ, D], F32, kind="ExternalOutput")

    with tile.TileContext(nc) as tc, ExitStack() as top:
        dram = top.enter_context(tc.tile_pool(name="dram", bufs=1, space="DRAM"))
        a2a_in = [dram.tile([NC, HD, SQ], BF16, name=f"a2ai{h}") for h in range(HPC)]
        a2a_out = [dram.tile([NC, HD, SQ], BF16, name=f"a2ao{h}") for h in range(HPC)]
        sync_in = dram.tile([NC, 1, 8], BF16, name="syncin")
        sync_out = dram.tile([NC, 1, 8], BF16, name="syncout")

        # constants via the software-DGE queue (gpsimd): keeps both HWDGE
        # queues clear for the startup X/W stream; none are needed before
        # ~20us in.
        const = top.enter_context(tc.tile_pool(name="const", bufs=1))
        r_sb = const.tile([128, 128], F32R)
        nc.gpsimd.dma_start(r_sb[:], RMAT.ap())
        idn_sb = const.tile([128, 128], BF16)
        nc.gpsimd.dma_start(idn_sb[:], IDN.ap())
        onesc_sb = const.tile([128, 1], BF16)
        nc.gpsimd.dma_start(onesc_sb[:], ONESC.ap())
        onesr_sb = const.tile([1, 128], F32R)
        nc.gpsimd.dma_start(onesr_sb[:], ONESR.ap())

        # Persistent Q^T/K^T ([hd, s], RoPE'd) and V ([s%128, s//128, hd]).
        res = top.enter_context(tc.tile_pool(name="res", bufs=1))
        qk_res = [res.tile([128, S], BF16, name=f"qk{m}") for m in range(8)]
        v_res = [res.tile([128, S // 128, 128], BF16, name=f"v{h}") for h in range(HPC)]

        XT_t = XT.ap().rearrange("(k p) s -> p k s", p=128)

        # ---- Phase 1: QKV projections + RoPE (Q,K) + transpose (V) ----
        # s-chunk outer (4 x 512 cols), m-tile inner; the full K=4096
        # contraction accumulates in one PSUM bank (32 matmuls).  X streams
        # once (double-buffered 4MB chunks), W streams once per chunk.
        with nc.named_scope("proj"), ExitStack() as ph:
            xt_pool = ph.enter_context(tc.tile_pool(name="xt", bufs=2))
            w_pool = ph.enter_context(tc.tile_pool(name="w", bufs=4))
            cs_pool = ph.enter_context(tc.tile_pool(name="cossin", bufs=2))
            rtmp = ph.enter_context(tc.tile_pool(name="rtmp", bufs=2))
            full_pool = ph.enter_context(tc.tile_pool(name="full", bufs=2))
            ppsum = ph.enter_context(tc.tile_pool(name="ppsum", bufs=4, space="PSUM"))
            rpsum = ph.enter_context(tc.tile_pool(name="rpsum", bufs=2, space="PSUM"))

            def load_cs(a):
                # scalar-queue DMAs: second HWDGE queue, decoupled from the
                # W-slab stream on the sync queue (ACT has slack in proj).
                cst = {}
                for nm, tt in (("cq", COSQ), ("sq", SINQ), ("ck", COSK), ("sk", SINK)):
                    t = cs_pool.tile([128, 512], F32, name=nm)
                    nc.scalar.dma_start(t[:], tt.ap()[:, a * 512 : (a + 1) * 512])
                    cst[nm] = t
                return cst

            def evacuate(m, a, ps, cst):
                s0 = a * 512
                sl = slice(s0, s0 + 512)
                if m < 8:  # Q or K m-tile: RoPE -> qk_res[m]
                    isq = m < 4
                    cos_t = cst["cq" if isq else "ck"]
                    sin_t = cst["sq" if isq else "sk"]
                    full_sb = full_pool.tile([128, 512], F32R, name="full")
                    nc.scalar.copy(full_sb[:], ps[:])
                    # rotate-half as a partition swap via SBUF->SBUF DMA
                    # (sign is folded into SINQ/SINK host-side) — keeps the
                    # rotation off the PE (was a 548ns fp32 matmul each).
                    rot_sb = rtmp.tile([128, 512], F32R, name="rot")
                    nc.scalar.dma_start(rot_sb[0:64, :], full_sb[64:128, :])
                    nc.sync.dma_start(rot_sb[64:128, :], full_sb[0:64, :])
                    c1 = rtmp.tile([128, 512], F32, name="c1")
                    nc.vector.tensor_mul(c1[:], full_sb[:], cos_t[:])
                    r1 = rtmp.tile([128, 512], F32, name="r1")
                    nc.vector.tensor_mul(r1[:], rot_sb[:], sin_t[:])
                    nc.vector.tensor_add(qk_res[m][:, sl], c1[:], r1[:])
                else:  # V m-tile: transpose to [s, hd] -> v_res[h]
                    h = m - 8
                    v_full = full_pool.tile([128, 512], BF16, name="vfull")
                    nc.scalar.copy(v_full[:], ps[:])
                    tps = rpsum.tile([128, 512], BF16, name="vtr")
                    for j in range(4):
                        jj = slice(j * 128, (j + 1) * 128)
                        nc.tensor.transpose(tps[:, jj], v_full[:, jj], idn_sb[:])
                    nc.vector.tensor_copy(v_res[h][:, 4 * a : 4 * a + 4, :], tps[:])

            def load_w(m):
                w_sb = w_pool.tile([128, KT, 128], BF16, name="w")
                nc.sync.dma_start(w_sb[:], W4.ap()[m])
                return w_sb

            # Startup interleave: first W slab goes out first (the first
            # matmul chain needs it plus only the first X piece), X pieces
            # pace the first chain, later W slabs slot between pieces.
            kq = KT // 8
            xt_cur = xt_pool.tile([128, KT, 512], BF16, name="xt")

            def xt0_piece(s, eng):
                eng.dma_start(xt_cur[:, s * kq : (s + 1) * kq, :],
                              XT_t[:, s * kq : (s + 1) * kq, 0:512])

            # startup: chunk-0 pieces split across both HWDGE queues so the
            # 4MB load lands in ~half the serial time; W slabs interleave
            # on the sync queue.
            w_pre = [load_w(0)]
            xt0_piece(0, nc.scalar)
            xt0_piece(1, nc.sync)
            xt0_piece(2, nc.scalar)
            w_pre.append(load_w(4))
            xt0_piece(4, nc.scalar)
            xt0_piece(3, nc.sync)
            w_pre.append(load_w(8))
            xt0_piece(5, nc.scalar)
            xt0_piece(6, nc.scalar)
            xt0_piece(7, nc.scalar)
            xt_next, cs_next = None, None
            cs_cur = load_cs(0)
            for a in range(4):
                for mi, m in enumerate((0, 4, 8, 1, 5, 9, 2, 6, 10, 3, 7, 11)):
                    w_sb = w_pre[mi] if (a == 0 and mi < 3) else load_w(m)
                    if a < 3:
                        # spread next-chunk prefetch: one 512KB piece per m
                        if mi == 2:
                            xt_next = xt_pool.tile([128, KT, 512], BF16,
                                                   name="xt")
                        if 2 <= mi < 10:
                            s = mi - 2
                            nc.scalar.dma_start(
                                xt_next[:, s * kq : (s + 1) * kq, :],
                                XT_t[:, s * kq : (s + 1) * kq,
                                     (a + 1) * 512 : (a + 2) * 512],
                            )
                        if mi == 10:
                            cs_next = load_cs(a + 1)
                    ps = ppsum.tile([128, 512], PF32, name="proj")
                    for kt in range(KT):
                        nc.tensor.matmul(ps[:], w_sb[:, kt, :], xt_cur[:, kt, :],
                                         start=(kt == 0), stop=(kt == KT - 1))
                    evacuate(m, a, ps, cs_cur)
                    if a == 0 and mi == 0:
                        # tiny collective doubling as a cross-core barrier:
                        # absorbs SPMD launch skew here, where the PE still
                        # has ~400us of queued work, so the first real
                        # AllToAll is not a skew sink.
                        zz = full_pool.tile([1, NC * 8], BF16, name="zz")
                        nc.any.memset(zz[:], 0.0)
                        nc.sync.dma_start(sync_in.rearrange("i p q -> p (i q)"),
                                          zz[:])
                        nc.gpsimd.collective_compute(
                            "AllToAll",
                            mybir.AluOpType.bypass,
                            replica_groups=[list(range(NC))],
                            ins=[sync_in.opt()],
                            outs=[sync_out.opt()],
                        )
                xt_cur, cs_cur = xt_next, cs_next

        # ---- Phase 2+3: attention interleaved with out-projection ----
        with nc.named_scope("attn"), ExitStack() as ph:
            es_pool = ph.enter_context(tc.tile_pool(name="es", bufs=6))
            on_pool = ph.enter_context(tc.tile_pool(name="on", bufs=4))
            of_pool = ph.enter_context(tc.tile_pool(name="of", bufs=2))
            wo_pool = ph.enter_context(tc.tile_pool(name="wo", bufs=6))
            yac_pool = ph.enter_context(tc.tile_pool(name="yac", bufs=1))
            yev_pool = ph.enter_context(tc.tile_pool(name="yev", bufs=4))
            msk_pool = ph.enter_context(tc.tile_pool(name="msk", bufs=1))
            spsum = ph.enter_context(tc.tile_pool(name="spsum", bufs=3, space="PSUM"))
            opsum = ph.enter_context(tc.tile_pool(name="opsum", bufs=2, space="PSUM"))
            cpsum = ph.enter_context(tc.tile_pool(name="cpsum", bufs=1, space="PSUM"))
            aux_ps = ph.enter_context(tc.tile_pool(name="auxps", bufs=2, space="PSUM"))

            masks_sb = msk_pool.tile([128, 4, 512], BF16, name="masks")
            nc.sync.dma_start(masks_sb[:], MASKS.ap())
            bias_sb = msk_pool.tile([128, D], F32, name="bias")
            nc.sync.dma_start(bias_sb[:], BOB.ap())
            yac = [yac_pool.tile([128, D], F32, name=f"yac{q2}") for q2 in range(2)]
            of_tiles = {}

            def attn_qr(h, qr):
                nk = 4 * (qr + 1)
                qsl = slice(qr * 512, (qr + 1) * 512)
                ops = opsum.tile([128, 512], PF32, name="ot")
                cps = cpsum.tile([1, 512], PF32, name="cs")
                for kt in range(nk):
                    sps = spsum.tile([128, 512], PF32, name="st")
                    nc.tensor.matmul(
                        sps[:], qk_res[4 + h][:, kt * 128 : (kt + 1) * 128],
                        qk_res[h][:, qsl], start=True, stop=True,
                    )
                    es = es_pool.tile([128, 512], BF16, name="es")
                    nc.scalar.activation(es[:], sps[:], AF.Exp)
                    if kt >= 4 * qr:
                        j = kt - 4 * qr
                        nc.vector.tensor_mul(es[:], es[:], masks_sb[:, j, :])
                    nc.tensor.matmul(ops[:], v_res[h][:, kt, :], es[:],
                                     start=(kt == 0), stop=(kt == nk - 1))
                    nc.tensor.matmul(cps[:], onesc_sb[:], es[:],
                                     start=(kt == 0), stop=(kt == nk - 1))
                # normalization: broadcast colsum across partitions, then
                # fast-approx reciprocal on all 128 lanes.
                cs_sb2 = on_pool.tile([1, 512], F32R, name="css")
                nc.vector.tensor_copy(cs_sb2[:], cps[:])
                bps = aux_ps.tile([128, 512], PF32, name="aux")
                nc.tensor.matmul(bps[:], onesr_sb[:], cs_sb2[:],
                                 start=True, stop=True)
                rec_sb = on_pool.tile([128, 512], F32, name="rec")
                nc.vector.reciprocal_approx_fast(out=rec_sb[:], in_=bps[:])
                otn = on_pool.tile([128, 512], BF16, name="otn")
                nc.vector.tensor_mul(otn[:], ops[:], rec_sb[:])
                for half in range(2):
                    nc.sync.dma_start(
                        a2a_in[h][2 * qr + half, :, :],
                        otn[:, half * 256 : (half + 1) * 256],
                    )

            def attn_head_end(h):
                nc.gpsimd.collective_compute(
                    "AllToAll",
                    mybir.AluOpType.bypass,
                    replica_groups=[list(range(NC))],
                    ins=[a2a_in[h].opt()],
                    outs=[a2a_out[h].opt()],
                )
                of = of_pool.tile([128, 8, SQ], BF16, name="of")
                nc.sync.dma_start(of[:], a2a_out[h].rearrange("i p q -> p i q"))
                of_tiles[h] = of

            def oproj_od(h, od):
                osl = slice(od * 512, (od + 1) * 512)
                wos = []
                for ih in range(2):
                    wo_sb = wo_pool.tile([128, 4, 512], BF16, name="wo")
                    # split the wo stream across both HWDGE queues: the sync
                    # queue alone starves the out-projection tail.
                    eng = nc.sync if ih == 0 else nc.scalar
                    eng.dma_start(
                        wo_sb[:],
                        WOT.ap()[h, od, ih * 4 : (ih + 1) * 4].rearrange(
                            "i p c -> p i c"
                        ),
                    )
                    wos.append(wo_sb)
                for q2 in range(2):
                    q2sl = slice(q2 * 128, (q2 + 1) * 128)
                    ps = aux_ps.tile([128, 512], PF32, name="aux")
                    for i in range(8):
                        nc.tensor.matmul(
                            ps[:], of_tiles[h][:, i, q2sl], wos[i // 4][:, i % 4, :],
                            start=(i == 0), stop=(i == 7),
                        )
                    if h == 0:
                        nc.vector.tensor_add(yac[q2][:, osl], ps[:],
                                             bias_sb[:, osl])
                    elif h < 3:
                        nc.vector.tensor_add(yac[q2][:, osl], yac[q2][:, osl],
                                             ps[:])
                    else:
                        y_sb = yev_pool.tile([128, 512], F32, name="y")
                        nc.vector.tensor_add(y_sb[:], ps[:], yac[q2][:, osl])
                        # Y stores ride the software-DGE queue: on a HWDGE
                        # queue they head-of-line block the od(3) wo loads
                        # (they wait on data that waits on the last a2a).
                        nc.gpsimd.dma_start(Y.ap()[q2sl, osl], y_sb[:])

            # qr largest-first.  Head h-1's out-projection slices start only
            # after qr3+qr2 of head h (~22us of compute cover for the
            # AllToAll, which doubles as a cross-core sync barrier), and two
            # slices trail past attn_head_end(h) so the next AllToAll always
            # has PE work to hide behind.
            # Out-projection lags the AllToAll by a FULL head (h-2 consumed
            # during head h): the Tile scheduler hoists every ready matmul,
            # so anything less leaves the oproj head-of-line-blocked on the
            # collective.  od slices of head 2 likewise cover a2a(3).
            for h in range(HPC):
                for i, qr in enumerate((3, 2, 1, 0)):
                    attn_qr(h, qr)
                    if h >= 2:
                        oproj_od(h - 2, 2 * i)
                        oproj_od(h - 2, 2 * i + 1)
                attn_head_end(h)
            for od in range(8):
                oproj_od(2, od)
            for od in range(8):
                oproj_od(3, od)

    nc.compile()
    return nc


def _prep_inputs(X, Wq, Wk, Wv, Wo, bo, cos, sin):
    import ml_dtypes
    BF = ml_dtypes.bfloat16

    X = np.asarray(X, dtype=np.float32)
    cos = np.asarray(cos, dtype=np.float32)
    sin = np.asarray(sin, dtype=np.float32)

    XTn = np.ascontiguousarray(X.reshape(S, D).T).astype(BF)   # [D, S]
    cosT = np.ascontiguousarray(cos.T)                         # [128, S]
    sinT = np.ascontiguousarray(sin.T).copy()
    # fold the rotate-half sign into sin: rope = x*cos + swap(x)*sin'
    # where swap is a pure partition exchange and sin'[:64] = -sin[:64].
    sinT[0:64, :] *= -1.0
    scale = np.float32(1.0 / np.sqrt(HD))
    COSQ, SINQ = cosT * scale, sinT * scale
    COSK, SINK = cosT, sinT

    R = np.zeros((128, 128), np.float32)
    for hd in range(64):
        R[hd + 64, hd] = -1.0     # rot[hd] = -x[hd+64]
        R[hd, hd + 64] = 1.0      # rot[hd+64] = x[hd]
    IDN = np.eye(128, dtype=np.float32).astype(BF)

    # masks[k, j, q] = 1 if (j*128 + k) <= q  (within a diagonal 512-block)
    kk = np.arange(128)[:, None, None]
    jj = np.arange(4)[None, :, None]
    qq = np.arange(512)[None, None, :]
    MASKS = ((jj * 128 + kk) <= qq).astype(np.float32).astype(BF)

    ONESC = np.ones((128, 1), np.float32).astype(BF)
    ONESR = np.ones((1, 128), np.float32)

    # [h, od, i, p, c] with global k-tile = 4*i + h (source core i, head h)
    WoT8 = np.ascontiguousarray(
        np.asarray(Wo, np.float32)
        .reshape(8, 512, 8, HPC, 128)
        .transpose(3, 0, 2, 4, 1)
    ).astype(BF)
    BOB = np.broadcast_to(np.asarray(bo, np.float32)[None, :], (128, D)).copy()

    shared = dict(
        XT=XTn, COSQ=COSQ, SINQ=SINQ, COSK=COSK, SINK=SINK, RMAT=R, IDN=IDN,
        MASKS=MASKS, ONESC=ONESC, ONESR=ONESR, WOT=WoT8, BOB=BOB,
    )
    in_maps = []
    for c in range(NC):
        lo, hi = c * 512, (c + 1) * 512
        Wcat = np.concatenate(
            [np.asarray(Wq, np.float32)[lo:hi],
             np.asarray(Wk, np.float32)[lo:hi],
             np.asarray(Wv, np.float32)[lo:hi]], axis=0
        )  # [1536, D]
        W4 = np.ascontiguousarray(
            Wcat.reshape(MPC, 128, KT, 128).transpose(0, 3, 2, 1)
        ).astype(BF)  # [m, p(k), k-tile, c(out)]
        in_maps.append({**shared, "W4": W4})
    return in_maps


def kernel(X, Wq, Wk, Wv, Wo, bo, cos, sin, _trace=False):
    from concourse.bass_utils import run_bass_kernel_spmd

    if "nc" not in _cache:
        _cache["nc"] = _build_program()
    nc = _cache["nc"]

    in_maps = _prep_inputs(X, Wq, Wk, Wv, Wo, bo, cos, sin)
    res = run_bass_kernel_spmd(nc, in_maps, list(range(NC)), trace=_trace)
    _cache["last_result"] = res
    Yfull = np.concatenate([res.results[c]["Y"] for c in range(NC)], axis=0)
    return Yfull.reshape(B, S, D).astype(np.float32)


# revision 43
# speedup vs baseline: 1.0339x; 1.0095x over previous
"""Tensor-parallel causal multi-head attention (RoPE) for 8 Trainium2 NeuronCores.

Problem: B=1, S=2048, D=4096, H=32 heads, head_dim=128, causal, RoPE,
out-projection with bias.  Reference: y = softmax(mask(QK^T/sqrt(hd))) V Wo^T + bo
with Q/K/V = X @ W{q,k,v}^T (nn.Linear convention) and RoPE applied to Q, K.

Sharding: tensor-parallel across heads (4 heads / core) for QKV + attention;
AllToAll re-shards to sequence (256 rows / core) for the out-projection.
Each core returns its 256-row slice of the final output; host concatenates.

v2 vs v1: all matmul operands in bf16 (fp32 PSUM accumulation; RoPE and
softmax normalization arithmetic in fp32 — measured end-to-end rel_l2 ~5e-3
vs the 2e-2 gate).  Q/K/V stay resident in SBUF (no DRAM spill round-trip).
Projections accumulate the full K=4096 contraction in PSUM (no SBUF
quarter-accumulation).  Softmax normalization uses a ones-broadcast matmul
followed by reciprocal_approx_fast (instead of a 3.4us full reciprocal).
Out-projection bias comes from a preloaded fp32 tile (saves 16 matmuls).
Attention q-chunks run largest-first so each head's AllToAll has ~12us of
compute cover before its out-projection consumes the result.
"""

import sys
import numpy as np

for _p in ("/opt/trn_rl_repo",):
    if _p not in sys.path:
        sys.path.insert(0, _p)

B, S, D, H = 1, 2048, 4096, 32
HD = 128          # head dim
NC = 8            # cores
HPC = H // NC     # heads per core = 4
MPC = 3 * HPC     # projection m-tiles per core (Q0..3, K0..3, V0..3) = 12
SQ = S // NC      # seq rows per core after AllToAll = 256
KT = D // 128     # contraction tiles = 32

_cache = {}


def _build_program():
    import concourse.bass as bass
    import concourse.mybir as mybir
    import concourse.tile as tile
    from concourse import bacc
    from contextlib import ExitStack

    F32 = mybir.dt.float32
    F32R = mybir.dt.float32r
    BF16 = mybir.dt.bfloat16
    PF32 = mybir.dt.float32
    AF = mybir.ActivationFunctionType

    nc = bacc.Bacc("TRN2", target_bir_lowering=False, debug=False, num_devices=NC)

    XT = nc.dram_tensor("XT", [D, S], BF16, kind="ExternalInput")
    W4 = nc.dram_tensor("W4", [MPC, 128, KT, 128], BF16, kind="ExternalInput")
    COSQ = nc.dram_tensor("COSQ", [128, S], F32, kind="ExternalInput")
    SINQ = nc.dram_tensor("SINQ", [128, S], F32, kind="ExternalInput")
    COSK = nc.dram_tensor("COSK", [128, S], F32, kind="ExternalInput")
    SINK = nc.dram_tensor("SINK", [128, S], F32, kind="ExternalInput")
    RMAT = nc.dram_tensor("RMAT", [128, 128], F32R, kind="ExternalInput")
    IDN = nc.dram_tensor("IDN", [128, 128], BF16, kind="ExternalInput")
    MASKS = nc.dram_tensor("MASKS", [128, 4, 512], BF16, kind="ExternalInput")
    ONESC = nc.dram_tensor("ONESC", [128, 1], BF16, kind="ExternalInput")
    ONESR = nc.dram_tensor("ONESR", [1, 128], F32R, kind="ExternalInput")
    WOT = nc.dram_tensor("WOT", [HPC, 8, 8, 128, 512], BF16, kind="ExternalInput")
    BOB = nc.dram_tensor("BOB", [128, D], F32, kind="ExternalInput")
    Y = nc.dram_tensor("Y", [SQ, D], F32, kind="ExternalOutput")

    with tile.TileContext(nc) as tc, ExitStack() as top:
        dram = top.enter_context(tc.tile_pool(name="dram", bufs=1, space="DRAM"))
        a2a_in = [dram.tile([NC, HD, SQ], BF16, name=f"a2ai{h}") for h in range(HPC)]
        a2a_out = [dram.tile([NC, HD, SQ], BF16, name=f"a2ao{h}") for h in range(HPC)]
        sync_in = dram.tile([NC, 1, 8], BF16, name="syncin")
        sync_out = dram.tile([NC, 1, 8], BF16, name="syncout")

        # constants via the software-DGE queue (gpsimd): keeps both HWDGE
        # queues clear for the startup X/W stream; none are needed before
        # ~20us in.
        const = top.enter_context(tc.tile_pool(name="const", bufs=1))
        r_sb = const.tile([128, 128], F32R)
        nc.gpsimd.dma_start(r_sb[:], RMAT.ap())
        idn_sb = const.tile([128, 128], BF16)
        nc.gpsimd.dma_start(idn_sb[:], IDN.ap())
        onesc_sb = const.tile([128, 1], BF16)
        nc.gpsimd.dma_start(onesc_sb[:], ONESC.ap())
        onesr_sb = const.tile([1, 128], F32R)
        nc.gpsimd.dma_start(onesr_sb[:], ONESR.ap())

        # Persistent Q^T/K^T ([hd, s], RoPE'd) and V ([s%128, s//128, hd]).
        res = top.enter_context(tc.tile_pool(name="res", bufs=1))
        qk_res = [res.tile([128, S], BF16, name=f"qk{m}") for m in range(8)]
        v_res = [res.tile([128, S // 128, 128], BF16, name=f"v{h}") for h in range(HPC)]

        XT_t = XT.ap().rearrange("(k p) s -> p k s", p=128)

        # ---- Phase 1: QKV projections + RoPE (Q,K) + transpose (V) ----
        # s-chunk outer (4 x 512 cols), m-tile inner; the full K=4096
        # contraction accumulates in one PSUM bank (32 matmuls).  X streams
        # once (double-buffered 4MB chunks), W streams once per chunk.
        with nc.named_scope("proj"), ExitStack() as ph:
            xt_pool = ph.enter_context(tc.tile_pool(name="xt", bufs=2))
            w_pool = ph.enter_context(tc.tile_pool(name="w", bufs=4))
            cs_pool = ph.enter_context(tc.tile_pool(name="cossin", bufs=2))
            rtmp = ph.enter_context(tc.tile_pool(name="rtmp", bufs=2))
            full_pool = ph.enter_context(tc.tile_pool(name="full", bufs=2))
            ppsum = ph.enter_context(tc.tile_pool(name="ppsum", bufs=4, space="PSUM"))
            rpsum = ph.enter_context(tc.tile_pool(name="rpsum", bufs=2, space="PSUM"))

            def load_cs(a):
                # scalar-queue DMAs: second HWDGE queue, decoupled from the
                # W-slab stream on the sync queue (ACT has slack in proj).
                cst = {}
                for nm, tt in (("cq", COSQ), ("sq", SINQ), ("ck", COSK), ("sk", SINK)):
                    t = cs_pool.tile([128, 512], F32, name=nm)
                    nc.scalar.dma_start(t[:], tt.ap()[:, a * 512 : (a + 1) * 512])
                    cst[nm] = t
                return cst

            def evacuate(m, a, ps, cst):
                s0 = a * 512
                sl = slice(s0, s0 + 512)
                if m < 8:  # Q or K m-tile: RoPE -> qk_res[m]
                    isq = m < 4
                    cos_t = cst["cq" if isq else "ck"]
                    sin_t = cst["sq" if isq else "sk"]
                    full_sb = full_pool.tile([128, 512], F32R, name="full")
                    nc.vector.tensor_copy(full_sb[:], ps[:])
                    # rotate-half as a partition swap via SBUF->SBUF DMA
                    # (sign is folded into SINQ/SINK host-side) — keeps the
                    # rotation off the PE (was a 548ns fp32 matmul each).
                    rot_sb = rtmp.tile([128, 512], F32R, name="rot")
                    nc.scalar.dma_start(rot_sb[0:64, :], full_sb[64:128, :])
                    nc.sync.dma_start(rot_sb[64:128, :], full_sb[0:64, :])
                    c1 = rtmp.tile([128, 512], F32, name="c1")
                    nc.vector.tensor_mul(c1[:], full_sb[:], cos_t[:])
                    r1 = rtmp.tile([128, 512], F32, name="r1")
                    nc.vector.tensor_mul(r1[:], rot_sb[:], sin_t[:])
                    nc.vector.tensor_add(qk_res[m][:, sl], c1[:], r1[:])
                else:  # V m-tile: transpose to [s, hd] -> v_res[h]
                    h = m - 8
                    v_full = full_pool.tile([128, 512], BF16, name="vfull")
                    nc.vector.tensor_copy(v_full[:], ps[:])
                    tps = rpsum.tile([128, 512], BF16, name="vtr")
                    for j in range(4):
                        jj = slice(j * 128, (j + 1) * 128)
                        nc.tensor.transpose(tps[:, jj], v_full[:, jj], idn_sb[:])
                    nc.vector.tensor_copy(v_res[h][:, 4 * a : 4 * a + 4, :], tps[:])

            def load_w(m):
                w_sb = w_pool.tile([128, KT, 128], BF16, name="w")
                nc.sync.dma_start(w_sb[:], W4.ap()[m])
                return w_sb

            # Startup interleave: first W slab goes out first (the first
            # matmul chain needs it plus only the first X piece), X pieces
            # pace the first chain, later W slabs slot between pieces.
            kq = KT // 8
            xt_cur = xt_pool.tile([128, KT, 512], BF16, name="xt")

            def xt0_piece(s, eng):
                eng.dma_start(xt_cur[:, s * kq : (s + 1) * kq, :],
                              XT_t[:, s * kq : (s + 1) * kq, 0:512])

            # startup: chunk-0 pieces split across both HWDGE queues so the
            # 4MB load lands in ~half the serial time; W slabs interleave
            # on the sync queue.
            w_pre = [load_w(0)]
            xt0_piece(0, nc.scalar)
            xt0_piece(1, nc.sync)
            xt0_piece(2, nc.scalar)
            w_pre.append(load_w(4))
            xt0_piece(4, nc.scalar)
            xt0_piece(3, nc.sync)
            w_pre.append(load_w(8))
            xt0_piece(5, nc.scalar)
            xt0_piece(6, nc.scalar)
            xt0_piece(7, nc.scalar)
            xt_next, cs_next = None, None
            cs_cur = load_cs(0)
            for a in range(4):
                for mi, m in enumerate((0, 4, 8, 1, 5, 9, 2, 6, 10, 3, 7, 11)):
                    w_sb = w_pre[mi] if (a == 0 and mi < 3) else load_w(m)
                    if a < 3:
                        # spread next-chunk prefetch: one 512KB piece per m
                        if mi == 2:
                            xt_next = xt_pool.tile([128, KT, 512], BF16,
                                                   name="xt")
                        if 2 <= mi < 10:
                            s = mi - 2
                            nc.scalar.dma_start(
                                xt_next[:, s * kq : (s + 1) * kq, :],
                                XT_t[:, s * kq : (s + 1) * kq,
                                     (a + 1) * 512 : (a + 2) * 512],
                            )
                        if mi == 10:
                            cs_next = load_cs(a + 1)
                    ps = ppsum.tile([128, 512], PF32, name="proj")
                    for kt in range(KT):
                        nc.tensor.matmul(ps[:], w_sb[:, kt, :], xt_cur[:, kt, :],
                                         start=(kt == 0), stop=(kt == KT - 1))
                    evacuate(m, a, ps, cs_cur)
                    if a == 0 and mi == 0:
                        # tiny collective doubling as a cross-core barrier:
                        # absorbs SPMD launch skew here, where the PE still
                        # has ~400us of queued work, so the first real
                        # AllToAll is not a skew sink.
                        zz = full_pool.tile([1, NC * 8], BF16, name="zz")
                        nc.any.memset(zz[:], 0.0)
                        nc.sync.dma_start(sync_in.rearrange("i p q -> p (i q)"),
                                          zz[:])
                        nc.gpsimd.collective_compute(
                            "AllToAll",
                            mybir.AluOpType.bypass,
                            replica_groups=[list(range(NC))],
                            ins=[sync_in.opt()],
                            outs=[sync_out.opt()],
                        )
                xt_cur, cs_cur = xt_next, cs_next

        # ---- Phase 2+3: attention interleaved with out-projection ----
        with nc.named_scope("attn"), ExitStack() as ph:
            es_pool = ph.enter_context(tc.tile_pool(name="es", bufs=6))
            on_pool = ph.enter_context(tc.tile_pool(name="on", bufs=4))
            of_pool = ph.enter_context(tc.tile_pool(name="of", bufs=2))
            wo_pool = ph.enter_context(tc.tile_pool(name="wo", bufs=8))
            yac_pool = ph.enter_context(tc.tile_pool(name="yac", bufs=1))
            yev_pool = ph.enter_context(tc.tile_pool(name="yev", bufs=4))
            msk_pool = ph.enter_context(tc.tile_pool(name="msk", bufs=1))
            spsum = ph.enter_context(tc.tile_pool(name="spsum", bufs=3, space="PSUM"))
            opsum = ph.enter_context(tc.tile_pool(name="opsum", bufs=2, space="PSUM"))
            cpsum = ph.enter_context(tc.tile_pool(name="cpsum", bufs=1, space="PSUM"))
            aux_ps = ph.enter_context(tc.tile_pool(name="auxps", bufs=2, space="PSUM"))

            masks_sb = msk_pool.tile([128, 4, 512], BF16, name="masks")
            nc.sync.dma_start(masks_sb[:], MASKS.ap())
            bias_sb = msk_pool.tile([128, D], F32, name="bias")
            nc.sync.dma_start(bias_sb[:], BOB.ap())
            yac = [yac_pool.tile([128, D], F32, name=f"yac{q2}") for q2 in range(2)]
            of_tiles = {}

            def attn_qr(h, qr):
                nk = 4 * (qr + 1)
                qsl = slice(qr * 512, (qr + 1) * 512)
                ops = opsum.tile([128, 512], PF32, name="ot")
                cps = cpsum.tile([1, 512], PF32, name="cs")
                es_prev = None
                for kt in range(nk):
                    sps = spsum.tile([128, 512], PF32, name="st")
                    nc.tensor.matmul(
                        sps[:], qk_res[4 + h][:, kt * 128 : (kt + 1) * 128],
                        qk_res[h][:, qsl], start=True, stop=True,
                    )
                    es = es_pool.tile([128, 512], BF16, name="es")
                    nc.scalar.activation(es[:], sps[:], AF.Exp)
                    if kt >= 4 * qr:
                        j = kt - 4 * qr
                        nc.vector.tensor_mul(es[:], es[:], masks_sb[:, j, :])
                    # colsum lags one k-tile: it reads an es that is already
                    # materialized, so it never stalls on exp and doubles as
                    # filler in front of the es-dependent PV matmul.
                    if es_prev is not None:
                        nc.tensor.matmul(cps[:], onesc_sb[:], es_prev[:],
                                         start=(kt == 1), stop=False)
                    nc.tensor.matmul(ops[:], v_res[h][:, kt, :], es[:],
                                     start=(kt == 0), stop=(kt == nk - 1))
                    es_prev = es
                nc.tensor.matmul(cps[:], onesc_sb[:], es_prev[:],
                                 start=False, stop=True)
                # normalization: broadcast colsum across partitions, then
                # fast-approx reciprocal on all 128 lanes.
                cs_sb2 = on_pool.tile([1, 512], F32R, name="css")
                nc.vector.tensor_copy(cs_sb2[:], cps[:])
                bps = aux_ps.tile([128, 512], PF32, name="aux")
                nc.tensor.matmul(bps[:], onesr_sb[:], cs_sb2[:],
                                 start=True, stop=True)
                rec_sb = on_pool.tile([128, 512], F32, name="rec")
                nc.vector.reciprocal_approx_fast(out=rec_sb[:], in_=bps[:])
                otn = on_pool.tile([128, 512], BF16, name="otn")
                nc.vector.tensor_mul(otn[:], ops[:], rec_sb[:])
                for half in range(2):
                    nc.sync.dma_start(
                        a2a_in[h][2 * qr + half, :, :],
                        otn[:, half * 256 : (half + 1) * 256],
                    )

            def attn_head_end(h):
                nc.gpsimd.collective_compute(
                    "AllToAll",
                    mybir.AluOpType.bypass,
                    replica_groups=[list(range(NC))],
                    ins=[a2a_in[h].opt()],
                    outs=[a2a_out[h].opt()],
                )
                of = of_pool.tile([128, 8, SQ], BF16, name="of")
                nc.sync.dma_start(of[:], a2a_out[h].rearrange("i p q -> p i q"))
                of_tiles[h] = of

            def oproj_od(h, od):
                osl = slice(od * 512, (od + 1) * 512)
                wos = []
                for ih in range(2):
                    wo_sb = wo_pool.tile([128, 4, 512], BF16, name="wo")
                    # split the wo stream across both HWDGE queues: the sync
                    # queue alone starves the out-projection tail.
                    eng = nc.sync if ih == 0 else nc.scalar
                    eng.dma_start(
                        wo_sb[:],
                        WOT.ap()[h, od, ih * 4 : (ih + 1) * 4].rearrange(
                            "i p c -> p i c"
                        ),
                    )
                    wos.append(wo_sb)
                for q2 in range(2):
                    q2sl = slice(q2 * 128, (q2 + 1) * 128)
                    ps = aux_ps.tile([128, 512], PF32, name="aux")
                    for i in range(8):
                        nc.tensor.matmul(
                            ps[:], of_tiles[h][:, i, q2sl], wos[i // 4][:, i % 4, :],
                            start=(i == 0), stop=(i == 7),
                        )
                    if h == 0:
                        nc.vector.tensor_add(yac[q2][:, osl], ps[:],
                                             bias_sb[:, osl])
                    elif h < 3:
                        nc.vector.tensor_add(yac[q2][:, osl], yac[q2][:, osl],
                                             ps[:])
                    else:
                        y_sb = yev_pool.tile([128, 512], F32, name="y")
                        nc.vector.tensor_add(y_sb[:], ps[:], yac[q2][:, osl])
                        # Y stores ride the software-DGE queue: on a HWDGE
                        # queue they head-of-line block the od(3) wo loads
                        # (they wait on data that waits on the last a2a).
                        nc.gpsimd.dma_start(Y.ap()[q2sl, osl], y_sb[:])

            # qr largest-first.  Head h-1's out-projection slices start only
            # after qr3+qr2 of head h (~22us of compute cover for the
            # AllToAll, which doubles as a cross-core sync barrier), and two
            # slices trail past attn_head_end(h) so the next AllToAll always
            # has PE work to hide behind.
            # Out-projection lags the AllToAll by a FULL head (h-2 consumed
            # during head h): the Tile scheduler hoists every ready matmul,
            # so anything less leaves the oproj head-of-line-blocked on the
            # collective.  od slices of head 2 likewise cover a2a(3).
            for h in range(HPC):
                for i, qr in enumerate((3, 2, 1, 0)):
                    attn_qr(h, qr)
                    if h >= 2:
                        oproj_od(h - 2, 2 * i)
                        oproj_od(h - 2, 2 * i + 1)
                attn_head_end(h)
            for od in range(8):
                oproj_od(2, od)
            for od in range(8):
                oproj_od(3, od)

    nc.compile()
    return nc


def _prep_inputs(X, Wq, Wk, Wv, Wo, bo, cos, sin):
    import ml_dtypes
    BF = ml_dtypes.bfloat16

    X = np.asarray(X, dtype=np.float32)
    cos = np.asarray(cos, dtype=np.float32)
    sin = np.asarray(sin, dtype=np.float32)

    XTn = np.ascontiguousarray(X.reshape(S, D).T).astype(BF)   # [D, S]
    cosT = np.ascontiguousarray(cos.T)                         # [128, S]
    sinT = np.ascontiguousarray(sin.T).copy()
    # fold the rotate-half sign into sin: rope = x*cos + swap(x)*sin'
    # where swap is a pure partition exchange and sin'[:64] = -sin[:64].
    sinT[0:64, :] *= -1.0
    scale = np.float32(1.0 / np.sqrt(HD))
    COSQ, SINQ = cosT * scale, sinT * scale
    COSK, SINK = cosT, sinT

    R = np.zeros((128, 128), np.float32)
    for hd in range(64):
        R[hd + 64, hd] = -1.0     # rot[hd] = -x[hd+64]
        R[hd, hd + 64] = 1.0      # rot[hd+64] = x[hd]
    IDN = np.eye(128, dtype=np.float32).astype(BF)

    # masks[k, j, q] = 1 if (j*128 + k) <= q  (within a diagonal 512-block)
    kk = np.arange(128)[:, None, None]
    jj = np.arange(4)[None, :, None]
    qq = np.arange(512)[None, None, :]
    MASKS = ((jj * 128 + kk) <= qq).astype(np.float32).astype(BF)

    ONESC = np.ones((128, 1), np.float32).astype(BF)
    ONESR = np.ones((1, 128), np.float32)

    # [h, od, i, p, c] with global k-tile = 4*i + h (source core i, head h)
    WoT8 = np.ascontiguousarray(
        np.asarray(Wo, np.float32)
        .reshape(8, 512, 8, HPC, 128)
        .transpose(3, 0, 2, 4, 1)
    ).astype(BF)
    BOB = np.broadcast_to(np.asarray(bo, np.float32)[None, :], (128, D)).copy()

    shared = dict(
        XT=XTn, COSQ=COSQ, SINQ=SINQ, COSK=COSK, SINK=SINK, RMAT=R, IDN=IDN,
        MASKS=MASKS, ONESC=ONESC, ONESR=ONESR, WOT=WoT8, BOB=BOB,
    )
    in_maps = []
    for c in range(NC):
        lo, hi = c * 512, (c + 1) * 512
        Wcat = np.concatenate(
            [np.asarray(Wq, np.float32)[lo:hi],
             np.asarray(Wk, np.float32)[lo:hi],
             np.asarray(Wv, np.float32)[lo:hi]], axis=0
        )  # [1536, D]
        W4 = np.ascontiguousarray(
            Wcat.reshape(MPC, 128, KT, 128).transpose(0, 3, 2, 1)
        ).astype(BF)  # [m, p(k), k-tile, c(out)]
        in_maps.append({**shared, "W4": W4})
    return in_maps


def kernel(X, Wq, Wk, Wv, Wo, bo, cos, sin, _trace=False):
    from concourse.bass_utils import run_bass_kernel_spmd

    if "nc" not in _cache:
        _cache["nc"] = _build_program()
    nc = _cache["nc"]

    in_maps = _prep_inputs(X, Wq, Wk, Wv, Wo, bo, cos, sin)
    res = run_bass_kernel_spmd(nc, in_maps, list(range(NC)), trace=_trace)
    _cache["last_result"] = res
    Yfull = np.concatenate([res.results[c]["Y"] for c in range(NC)], axis=0)
    return Yfull.reshape(B, S, D).astype(np.float32)


# revision 44
# speedup vs baseline: 1.0363x; 1.0023x over previous
"""Tensor-parallel causal multi-head attention (RoPE) for 8 Trainium2 NeuronCores.

Problem: B=1, S=2048, D=4096, H=32 heads, head_dim=128, causal, RoPE,
out-projection with bias.  Reference: y = softmax(mask(QK^T/sqrt(hd))) V Wo^T + bo
with Q/K/V = X @ W{q,k,v}^T (nn.Linear convention) and RoPE applied to Q, K.

Sharding: tensor-parallel across heads (4 heads / core) for QKV + attention;
AllToAll re-shards to sequence (256 rows / core) for the out-projection.
Each core returns its 256-row slice of the final output; host concatenates.

v2 vs v1: all matmul operands in bf16 (fp32 PSUM accumulation; RoPE and
softmax normalization arithmetic in fp32 — measured end-to-end rel_l2 ~5e-3
vs the 2e-2 gate).  Q/K/V stay resident in SBUF (no DRAM spill round-trip).
Projections accumulate the full K=4096 contraction in PSUM (no SBUF
quarter-accumulation).  Softmax normalization uses a ones-broadcast matmul
followed by reciprocal_approx_fast (instead of a 3.4us full reciprocal).
Out-projection bias comes from a preloaded fp32 tile (saves 16 matmuls).
Attention q-chunks run largest-first so each head's AllToAll has ~12us of
compute cover before its out-projection consumes the result.
"""

import sys
import numpy as np

for _p in ("/opt/trn_rl_repo",):
    if _p not in sys.path:
        sys.path.insert(0, _p)

B, S, D, H = 1, 2048, 4096, 32
HD = 128          # head dim
NC = 8            # cores
HPC = H // NC     # heads per core = 4
MPC = 3 * HPC     # projection m-tiles per core (Q0..3, K0..3, V0..3) = 12
SQ = S // NC      # seq rows per core after AllToAll = 256
KT = D // 128     # contraction tiles = 32

_cache = {}


def _build_program():
    import concourse.bass as bass
    import concourse.mybir as mybir
    import concourse.tile as tile
    from concourse import bacc
    from contextlib import ExitStack

    F32 = mybir.dt.float32
    F32R = mybir.dt.float32r
    BF16 = mybir.dt.bfloat16
    PF32 = mybir.dt.float32
    AF = mybir.ActivationFunctionType

    nc = bacc.Bacc("TRN2", target_bir_lowering=False, debug=False, num_devices=NC)

    XT = nc.dram_tensor("XT", [D, S], BF16, kind="ExternalInput")
    W4 = nc.dram_tensor("W4", [MPC, 128, KT, 128], BF16, kind="ExternalInput")
    COSQ = nc.dram_tensor("COSQ", [128, S], F32, kind="ExternalInput")
    SINQ = nc.dram_tensor("SINQ", [128, S], F32, kind="ExternalInput")
    COSK = nc.dram_tensor("COSK", [128, S], F32, kind="ExternalInput")
    SINK = nc.dram_tensor("SINK", [128, S], F32, kind="ExternalInput")
    RMAT = nc.dram_tensor("RMAT", [128, 128], F32R, kind="ExternalInput")
    IDN = nc.dram_tensor("IDN", [128, 128], BF16, kind="ExternalInput")
    MASKS = nc.dram_tensor("MASKS", [128, 4, 512], BF16, kind="ExternalInput")
    ONESC = nc.dram_tensor("ONESC", [128, 1], BF16, kind="ExternalInput")
    ONESR = nc.dram_tensor("ONESR", [1, 128], F32R, kind="ExternalInput")
    WOT = nc.dram_tensor("WOT", [HPC, 8, 8, 128, 512], BF16, kind="ExternalInput")
    BOB = nc.dram_tensor("BOB", [128, D], F32, kind="ExternalInput")
    Y = nc.dram_tensor("Y", [SQ, D], F32, kind="ExternalOutput")

    with tile.TileContext(nc) as tc, ExitStack() as top:
        dram = top.enter_context(tc.tile_pool(name="dram", bufs=1, space="DRAM"))
        a2a_in = [dram.tile([NC, HD, SQ], BF16, name=f"a2ai{h}") for h in range(HPC)]
        a2a_out = [dram.tile([NC, HD, SQ], BF16, name=f"a2ao{h}") for h in range(HPC)]
        sync_in = dram.tile([NC, 1, 8], BF16, name="syncin")
        sync_out = dram.tile([NC, 1, 8], BF16, name="syncout")

        # constants via the software-DGE queue (gpsimd): keeps both HWDGE
        # queues clear for the startup X/W stream; none are needed before
        # ~20us in.
        const = top.enter_context(tc.tile_pool(name="const", bufs=1))
        r_sb = const.tile([128, 128], F32R)
        nc.gpsimd.dma_start(r_sb[:], RMAT.ap())
        idn_sb = const.tile([128, 128], BF16)
        nc.gpsimd.dma_start(idn_sb[:], IDN.ap())
        onesc_sb = const.tile([128, 1], BF16)
        nc.gpsimd.dma_start(onesc_sb[:], ONESC.ap())
        onesr_sb = const.tile([1, 128], F32R)
        nc.gpsimd.dma_start(onesr_sb[:], ONESR.ap())

        # Persistent Q^T/K^T ([hd, s], RoPE'd) and V ([s%128, s//128, hd]).
        res = top.enter_context(tc.tile_pool(name="res", bufs=1))
        qk_res = [res.tile([128, S], BF16, name=f"qk{m}") for m in range(8)]
        v_res = [res.tile([128, S // 128, 128], BF16, name=f"v{h}") for h in range(HPC)]

        XT_t = XT.ap().rearrange("(k p) s -> p k s", p=128)

        # ---- Phase 1: QKV projections + RoPE (Q,K) + transpose (V) ----
        # s-chunk outer (4 x 512 cols), m-tile inner; the full K=4096
        # contraction accumulates in one PSUM bank (32 matmuls).  X streams
        # once (double-buffered 4MB chunks), W streams once per chunk.
        with nc.named_scope("proj"), ExitStack() as ph:
            xt_pool = ph.enter_context(tc.tile_pool(name="xt", bufs=2))
            w_pool = ph.enter_context(tc.tile_pool(name="w", bufs=4))
            cs_pool = ph.enter_context(tc.tile_pool(name="cossin", bufs=2))
            rtmp = ph.enter_context(tc.tile_pool(name="rtmp", bufs=2))
            full_pool = ph.enter_context(tc.tile_pool(name="full", bufs=2))
            ppsum = ph.enter_context(tc.tile_pool(name="ppsum", bufs=4, space="PSUM"))
            rpsum = ph.enter_context(tc.tile_pool(name="rpsum", bufs=2, space="PSUM"))

            def load_cs(a):
                # scalar-queue DMAs: second HWDGE queue, decoupled from the
                # W-slab stream on the sync queue (ACT has slack in proj).
                cst = {}
                for nm, tt in (("cq", COSQ), ("sq", SINQ), ("ck", COSK), ("sk", SINK)):
                    t = cs_pool.tile([128, 512], F32, name=nm)
                    nc.scalar.dma_start(t[:], tt.ap()[:, a * 512 : (a + 1) * 512])
                    cst[nm] = t
                return cst

            def evacuate(m, a, ps, cst):
                s0 = a * 512
                sl = slice(s0, s0 + 512)
                if m < 8:  # Q or K m-tile: RoPE -> qk_res[m]
                    isq = m < 4
                    cos_t = cst["cq" if isq else "ck"]
                    sin_t = cst["sq" if isq else "sk"]
                    full_sb = full_pool.tile([128, 512], F32R, name="full")
                    nc.vector.tensor_copy(full_sb[:], ps[:])
                    # rotate-half as a partition swap via SBUF->SBUF DMA
                    # (sign is folded into SINQ/SINK host-side) — keeps the
                    # rotation off the PE (was a 548ns fp32 matmul each).
                    rot_sb = rtmp.tile([128, 512], F32R, name="rot")
                    nc.scalar.dma_start(rot_sb[0:64, :], full_sb[64:128, :])
                    nc.sync.dma_start(rot_sb[64:128, :], full_sb[0:64, :])
                    c1 = rtmp.tile([128, 512], F32, name="c1")
                    nc.vector.tensor_mul(c1[:], full_sb[:], cos_t[:])
                    r1 = rtmp.tile([128, 512], F32, name="r1")
                    nc.vector.tensor_mul(r1[:], rot_sb[:], sin_t[:])
                    nc.vector.tensor_add(qk_res[m][:, sl], c1[:], r1[:])
                else:  # V m-tile: transpose to [s, hd] -> v_res[h]
                    h = m - 8
                    v_full = full_pool.tile([128, 512], BF16, name="vfull")
                    nc.vector.tensor_copy(v_full[:], ps[:])
                    tps = rpsum.tile([128, 512], BF16, name="vtr")
                    for j in range(4):
                        jj = slice(j * 128, (j + 1) * 128)
                        nc.tensor.transpose(tps[:, jj], v_full[:, jj], idn_sb[:])
                    nc.vector.tensor_copy(v_res[h][:, 4 * a : 4 * a + 4, :], tps[:])

            def load_w(m):
                w_sb = w_pool.tile([128, KT, 128], BF16, name="w")
                nc.sync.dma_start(w_sb[:], W4.ap()[m])
                return w_sb

            # Startup interleave: first W slab goes out first (the first
            # matmul chain needs it plus only the first X piece), X pieces
            # pace the first chain, later W slabs slot between pieces.
            kq = KT // 8
            xt_cur = xt_pool.tile([128, KT, 512], BF16, name="xt")

            def xt0_piece(s, eng):
                eng.dma_start(xt_cur[:, s * kq : (s + 1) * kq, :],
                              XT_t[:, s * kq : (s + 1) * kq, 0:512])

            # startup: chunk-0 pieces split across both HWDGE queues so the
            # 4MB load lands in ~half the serial time; W slabs interleave
            # on the sync queue.
            w_pre = [load_w(0)]
            xt0_piece(0, nc.scalar)
            xt0_piece(1, nc.sync)
            xt0_piece(2, nc.scalar)
            w_pre.append(load_w(4))
            xt0_piece(4, nc.scalar)
            xt0_piece(3, nc.sync)
            w_pre.append(load_w(8))
            xt0_piece(5, nc.scalar)
            xt0_piece(6, nc.scalar)
            xt0_piece(7, nc.scalar)
            xt_next, cs_next = None, None
            cs_cur = load_cs(0)
            for a in range(4):
                for mi, m in enumerate((0, 4, 8, 1, 5, 9, 2, 6, 10, 3, 7, 11)):
                    w_sb = w_pre[mi] if (a == 0 and mi < 3) else load_w(m)
                    if a < 3:
                        # spread next-chunk prefetch: one 512KB piece per m
                        if mi == 2:
                            xt_next = xt_pool.tile([128, KT, 512], BF16,
                                                   name="xt")
                        if 2 <= mi < 10:
                            s = mi - 2
                            nc.scalar.dma_start(
                                xt_next[:, s * kq : (s + 1) * kq, :],
                                XT_t[:, s * kq : (s + 1) * kq,
                                     (a + 1) * 512 : (a + 2) * 512],
                            )
                        if mi == 10:
                            cs_next = load_cs(a + 1)
                    ps = ppsum.tile([128, 512], PF32, name="proj")
                    for kt in range(KT):
                        nc.tensor.matmul(ps[:], w_sb[:, kt, :], xt_cur[:, kt, :],
                                         start=(kt == 0), stop=(kt == KT - 1))
                    evacuate(m, a, ps, cs_cur)
                    if a == 0 and mi == 0:
                        # tiny collective doubling as a cross-core barrier:
                        # absorbs SPMD launch skew here, where the PE still
                        # has ~400us of queued work, so the first real
                        # AllToAll is not a skew sink.
                        zz = full_pool.tile([1, NC * 8], BF16, name="zz")
                        nc.any.memset(zz[:], 0.0)
                        nc.sync.dma_start(sync_in.rearrange("i p q -> p (i q)"),
                                          zz[:])
                        nc.gpsimd.collective_compute(
                            "AllToAll",
                            mybir.AluOpType.bypass,
                            replica_groups=[list(range(NC))],
                            ins=[sync_in.opt()],
                            outs=[sync_out.opt()],
                        )
                xt_cur, cs_cur = xt_next, cs_next

        # ---- Phase 2+3: attention interleaved with out-projection ----
        with nc.named_scope("attn"), ExitStack() as ph:
            es_pool = ph.enter_context(tc.tile_pool(name="es", bufs=8))
            on_pool = ph.enter_context(tc.tile_pool(name="on", bufs=4))
            of_pool = ph.enter_context(tc.tile_pool(name="of", bufs=2))
            wo_pool = ph.enter_context(tc.tile_pool(name="wo", bufs=8))
            yac_pool = ph.enter_context(tc.tile_pool(name="yac", bufs=1))
            yev_pool = ph.enter_context(tc.tile_pool(name="yev", bufs=4))
            msk_pool = ph.enter_context(tc.tile_pool(name="msk", bufs=1))
            spsum = ph.enter_context(tc.tile_pool(name="spsum", bufs=3, space="PSUM"))
            opsum = ph.enter_context(tc.tile_pool(name="opsum", bufs=2, space="PSUM"))
            cpsum = ph.enter_context(tc.tile_pool(name="cpsum", bufs=1, space="PSUM"))
            aux_ps = ph.enter_context(tc.tile_pool(name="auxps", bufs=2, space="PSUM"))

            masks_sb = msk_pool.tile([128, 4, 512], BF16, name="masks")
            nc.sync.dma_start(masks_sb[:], MASKS.ap())
            bias_sb = msk_pool.tile([128, D], F32, name="bias")
            nc.sync.dma_start(bias_sb[:], BOB.ap())
            yac = [yac_pool.tile([128, D], F32, name=f"yac{q2}") for q2 in range(2)]
            of_tiles = {}

            def attn_qr(h, qr):
                nk = 4 * (qr + 1)
                qsl = slice(qr * 512, (qr + 1) * 512)
                ops = opsum.tile([128, 512], PF32, name="ot")
                cps = cpsum.tile([1, 512], PF32, name="cs")
                es_prev = None
                for kt in range(nk):
                    sps = spsum.tile([128, 512], PF32, name="st")
                    nc.tensor.matmul(
                        sps[:], qk_res[4 + h][:, kt * 128 : (kt + 1) * 128],
                        qk_res[h][:, qsl], start=True, stop=True,
                    )
                    es = es_pool.tile([128, 512], BF16, name="es")
                    nc.scalar.activation(es[:], sps[:], AF.Exp)
                    if kt >= 4 * qr:
                        j = kt - 4 * qr
                        nc.vector.tensor_mul(es[:], es[:], masks_sb[:, j, :])
                    # colsum lags one k-tile: it reads an es that is already
                    # materialized, so it never stalls on exp and doubles as
                    # filler in front of the es-dependent PV matmul.
                    if es_prev is not None:
                        nc.tensor.matmul(cps[:], onesc_sb[:], es_prev[:],
                                         start=(kt == 1), stop=False)
                    nc.tensor.matmul(ops[:], v_res[h][:, kt, :], es[:],
                                     start=(kt == 0), stop=(kt == nk - 1))
                    es_prev = es
                nc.tensor.matmul(cps[:], onesc_sb[:], es_prev[:],
                                 start=False, stop=True)
                # normalization: broadcast colsum across partitions, then
                # fast-approx reciprocal on all 128 lanes.
                cs_sb2 = on_pool.tile([1, 512], F32R, name="css")
                nc.vector.tensor_copy(cs_sb2[:], cps[:])
                bps = aux_ps.tile([128, 512], PF32, name="aux")
                nc.tensor.matmul(bps[:], onesr_sb[:], cs_sb2[:],
                                 start=True, stop=True)
                rec_sb = on_pool.tile([128, 512], F32, name="rec")
                nc.vector.reciprocal_approx_fast(out=rec_sb[:], in_=bps[:])
                otn = on_pool.tile([128, 512], BF16, name="otn")
                nc.vector.tensor_mul(otn[:], ops[:], rec_sb[:])
                for half in range(2):
                    nc.sync.dma_start(
                        a2a_in[h][2 * qr + half, :, :],
                        otn[:, half * 256 : (half + 1) * 256],
                    )

            def attn_head_end(h):
                nc.gpsimd.collective_compute(
                    "AllToAll",
                    mybir.AluOpType.bypass,
                    replica_groups=[list(range(NC))],
                    ins=[a2a_in[h].opt()],
                    outs=[a2a_out[h].opt()],
                )
                of = of_pool.tile([128, 8, SQ], BF16, name="of")
                nc.sync.dma_start(of[:], a2a_out[h].rearrange("i p q -> p i q"))
                of_tiles[h] = of

            def oproj_od(h, od):
                osl = slice(od * 512, (od + 1) * 512)
                wos = []
                for ih in range(2):
                    wo_sb = wo_pool.tile([128, 4, 512], BF16, name="wo")
                    # split the wo stream across both HWDGE queues: the sync
                    # queue alone starves the out-projection tail.
                    eng = nc.sync if ih == 0 else nc.scalar
                    eng.dma_start(
                        wo_sb[:],
                        WOT.ap()[h, od, ih * 4 : (ih + 1) * 4].rearrange(
                            "i p c -> p i c"
                        ),
                    )
                    wos.append(wo_sb)
                for q2 in range(2):
                    q2sl = slice(q2 * 128, (q2 + 1) * 128)
                    ps = aux_ps.tile([128, 512], PF32, name="aux")
                    for i in range(8):
                        nc.tensor.matmul(
                            ps[:], of_tiles[h][:, i, q2sl], wos[i // 4][:, i % 4, :],
                            start=(i == 0), stop=(i == 7),
                        )
                    if h == 0:
                        nc.vector.tensor_add(yac[q2][:, osl], ps[:],
                                             bias_sb[:, osl])
                    elif h < 3:
                        nc.vector.tensor_add(yac[q2][:, osl], yac[q2][:, osl],
                                             ps[:])
                    else:
                        y_sb = yev_pool.tile([128, 512], F32, name="y")
                        nc.vector.tensor_add(y_sb[:], ps[:], yac[q2][:, osl])
                        # Y stores ride the software-DGE queue: on a HWDGE
                        # queue they head-of-line block the od(3) wo loads
                        # (they wait on data that waits on the last a2a).
                        nc.gpsimd.dma_start(Y.ap()[q2sl, osl], y_sb[:])

            # qr largest-first.  Head h-1's out-projection slices start only
            # after qr3+qr2 of head h (~22us of compute cover for the
            # AllToAll, which doubles as a cross-core sync barrier), and two
            # slices trail past attn_head_end(h) so the next AllToAll always
            # has PE work to hide behind.
            # Out-projection lags the AllToAll by a FULL head (h-2 consumed
            # during head h): the Tile scheduler hoists every ready matmul,
            # so anything less leaves the oproj head-of-line-blocked on the
            # collective.  od slices of head 2 likewise cover a2a(3).
            for h in range(HPC):
                for i, qr in enumerate((3, 2, 1, 0)):
                    attn_qr(h, qr)
                    if h >= 2:
                        oproj_od(h - 2, 2 * i)
                        oproj_od(h - 2, 2 * i + 1)
                attn_head_end(h)
            for od in range(8):
                oproj_od(2, od)
            for od in range(8):
                oproj_od(3, od)

    nc.compile()
    return nc


def _prep_inputs(X, Wq, Wk, Wv, Wo, bo, cos, sin):
    import ml_dtypes
    BF = ml_dtypes.bfloat16

    X = np.asarray(X, dtype=np.float32)
    cos = np.asarray(cos, dtype=np.float32)
    sin = np.asarray(sin, dtype=np.float32)

    XTn = np.ascontiguousarray(X.reshape(S, D).T).astype(BF)   # [D, S]
    cosT = np.ascontiguousarray(cos.T)                         # [128, S]
    sinT = np.ascontiguousarray(sin.T).copy()
    # fold the rotate-half sign into sin: rope = x*cos + swap(x)*sin'
    # where swap is a pure partition exchange and sin'[:64] = -sin[:64].
    sinT[0:64, :] *= -1.0
    scale = np.float32(1.0 / np.sqrt(HD))
    COSQ, SINQ = cosT * scale, sinT * scale
    COSK, SINK = cosT, sinT

    R = np.zeros((128, 128), np.float32)
    for hd in range(64):
        R[hd + 64, hd] = -1.0     # rot[hd] = -x[hd+64]
        R[hd, hd + 64] = 1.0      # rot[hd+64] = x[hd]
    IDN = np.eye(128, dtype=np.float32).astype(BF)

    # masks[k, j, q] = 1 if (j*128 + k) <= q  (within a diagonal 512-block)
    kk = np.arange(128)[:, None, None]
    jj = np.arange(4)[None, :, None]
    qq = np.arange(512)[None, None, :]
    MASKS = ((jj * 128 + kk) <= qq).astype(np.float32).astype(BF)

    ONESC = np.ones((128, 1), np.float32).astype(BF)
    ONESR = np.ones((1, 128), np.float32)

    # [h, od, i, p, c] with global k-tile = 4*i + h (source core i, head h)
    WoT8 = np.ascontiguousarray(
        np.asarray(Wo, np.float32)
        .reshape(8, 512, 8, HPC, 128)
        .transpose(3, 0, 2, 4, 1)
    ).astype(BF)
    BOB = np.broadcast_to(np.asarray(bo, np.float32)[None, :], (128, D)).copy()

    shared = dict(
        XT=XTn, COSQ=COSQ, SINQ=SINQ, COSK=COSK, SINK=SINK, RMAT=R, IDN=IDN,
        MASKS=MASKS, ONESC=ONESC, ONESR=ONESR, WOT=WoT8, BOB=BOB,
    )
    in_maps = []
    for c in range(NC):
        lo, hi = c * 512, (c + 1) * 512
        Wcat = np.concatenate(
            [np.asarray(Wq, np.float32)[lo:hi],
             np.asarray(Wk, np.float32)[lo:hi],
             np.asarray(Wv, np.float32)[lo:hi]], axis=0
        )  # [1536, D]
        W4 = np.ascontiguousarray(
            Wcat.reshape(MPC, 128, KT, 128).transpose(0, 3, 2, 1)
        ).astype(BF)  # [m, p(k), k-tile, c(out)]
        in_maps.append({**shared, "W4": W4})
    return in_maps


def kernel(X, Wq, Wk, Wv, Wo, bo, cos, sin, _trace=False):
    from concourse.bass_utils import run_bass_kernel_spmd

    if "nc" not in _cache:
        _cache["nc"] = _build_program()
    nc = _cache["nc"]

    in_maps = _prep_inputs(X, Wq, Wk, Wv, Wo, bo, cos, sin)
    res = run_bass_kernel_spmd(nc, in_maps, list(range(NC)), trace=_trace)
    _cache["last_result"] = res
    Yfull = np.concatenate([res.results[c]["Y"] for c in range(NC)], axis=0)
    return Yfull.reshape(B, S, D).astype(np.float32)
